# revision 1
# baseline (speedup 1.0000x reference)
"""Trainium2 kernel for CondensedLinearFineGrainedSparseOp:
    out[b,s,o] = sum_k x[b,s,k] * weight[o,k] + bias[o]
with x [8, 2048, 4096] f32, weight [4096, 4096] f32 (90% zeros, stored
dense), bias [4096] f32 -> out [8, 2048, 4096] f32.

Strategy: data-parallel shard over tokens (B*S = 16384 -> 2048 per core)
across 8 NeuronCores; weight/bias replicated. The unstructured 10%
sparsity is not exploitable on the 128x128 PE array (any >=8x8 block of
the mask is nonempty with overwhelming probability), so each core runs a
dense [2048 x 4096 x 4096] GEMM in bf16 with fp32 PSUM accumulation.

Per-core kernel: out[t,o] with t on PSUM partitions. Stationary operand =
x^T tile [128k, 128t]; moving operand = W^T tile [128k, 512o]. o is
processed in 4 blocks of 1024; each o-block's 32 W^T k-tiles are SBUF
resident (double-buffered across blocks, 128KB/partition) while x^T
streams per t-tile (re-read once per o-block). Bias is pre-replicated
across 128 partitions on host and added during PSUM->SBUF eviction.

HBM traffic/core ~130MB (~0.43ms) < PE dense compute ~0.87ms -> compute
bound at the bf16 PE roofline.
"""

import os

import numpy as np
import ml_dtypes

import concourse.mybir as mybir
import concourse.tile as tile
from concourse import bacc
from concourse.bass import ts
from concourse.bass_utils import run_bass_kernel_spmd

P = 128
NCORES = 8
B, S, DIN, DOUT = 8, 2048, 4096, 4096
T = B * S // NCORES          # tokens per core
KT = DIN // P                # 32 contraction tiles
NT = T // P                  # 16 token tiles per core
OBLK = 1024                  # steady-state o-block (SBUF-resident W slice)
# First block split 512+512: the cold-start W mass is halved so the PE
# starts ~15us in; later blocks prefetch behind compute. (512 is the
# minimum efficient moving width: at 256 the per-matmul LDWEIGHTS no
# longer hides behind the stream and throughput drops.)
PHASES = [(0, 512), (512, 512)] + [
    (o, OBLK) for o in range(OBLK, DOUT, OBLK)
]

BF16 = mybir.dt.bfloat16
F32 = mybir.dt.float32

_NC = None
LAST_RESULT = None


def _build_nc():
    nc = bacc.Bacc("TRN2", target_bir_lowering=False, debug=False)
    # x pre-tiled on host to the exact SBUF image of each t-tile:
    # xt[t, p, ks, i] = x[t*128+i, ks*128+p] -> each t-tile DMA is one
    # fully linear 1MB read (per-partition 8KB contiguous)
    xt = nc.dram_tensor("xt", [NT, P, KT, P], BF16, kind="ExternalInput")
    wt = nc.dram_tensor("wt", [DIN, DOUT], BF16, kind="ExternalInput")
    bias = nc.dram_tensor("bias_rep", [P, DOUT], F32, kind="ExternalInput")
    out = nc.dram_tensor("out", [T, DOUT], F32, kind="ExternalOutput")

    with tile.TileContext(nc) as tc:
        with (
            tc.tile_pool(name="wpool", bufs=2 * KT - 2) as wpool,
            tc.tile_pool(name="xpool", bufs=4) as xpool,
            tc.tile_pool(name="bpool", bufs=1) as bpool,
            tc.tile_pool(name="opool", bufs=4) as opool,
            tc.tile_pool(name="psum", bufs=8, space="PSUM") as psum_pool,
        ):
            # Tiny warmup DMA on each queue first: absorbs cold DGE/queue
            # init and first-completion latency on throwaway transfers
            # instead of the critical first W/x tiles.
            for i, eng in enumerate((nc.sync, nc.scalar, nc.gpsimd)):
                wu = bpool.tile([P, 8], F32, tag=f"wu{i}", name=f"wu{i}")
                eng.dma_start(wu[:], bias.ap()[:, ts(i, 8)])

            # First x tile in 4 chunks ahead of everything on the SWDGE
            # queue, so the PE's first stationary operand arrives within
            # ~10us even while the cold W stream saturates HBM.
            x_first = xpool.tile([P, KT, P], BF16, tag="x", name="x_first")
            for c in range(4):
                nc.gpsimd.dma_start(
                    x_first[:, ts(c, KT // 4), :],
                    xt.ap()[0, :, ts(c, KT // 4), :],
                )
            bias_sb = bpool.tile([P, DOUT], F32)

            for ph, (o0, olen) in enumerate(PHASES):
                banks = [olen] if olen < 512 else [512] * (olen // 512)
                # W^T k-tiles for this o-block; 2*KT slots in the pool
                # double-buffer the next block's stream behind this one.
                w_tiles = []
                for k in range(KT):
                    wtile = wpool.tile(
                        [P, olen], BF16, tag="w", name="w",
                        padded_shape=[P, OBLK],
                    )
                    # alternate the two HWDGE queues to double W stream
                    # issue rate (matters for the cold first block)
                    eng = nc.sync if k % 2 == 0 else nc.scalar
                    eng.dma_start(
                        wtile[:], wt.ap()[ts(k, P), o0 : o0 + olen]
                    )
                    w_tiles.append(wtile)

                if ph == 0:
                    # bias queued on sync behind phase-0's small W stream:
                    # off the cold-start critical path, but well before the
                    # first PSUM eviction needs it
                    nc.sync.dma_start(bias_sb[:], bias.ap())

                for t in range(NT):
                    if ph == 0 and t == 0:
                        xtile = x_first
                    else:
                        # x^T tile [p, ks, i]: one linear 1MB DMA on the
                        # SWDGE queue, decoupled from the W streams on the
                        # two HWDGE queues
                        xtile = xpool.tile([P, KT, P], BF16, tag="x")
                        nc.gpsimd.dma_start(xtile[:], xt.ap()[t])

                    accs = [
                        psum_pool.tile([P, blen], F32, tag="acc", name="acc",
                                       padded_shape=[P, 512])
                        for blen in banks
                    ]
                    for k in range(KT):
                        for b, blen in enumerate(banks):
                            nc.tensor.matmul(
                                accs[b][:],
                                xtile[:, k, :],                  # stationary
                                w_tiles[k][:, ts(b, blen)],      # moving
                                start=(k == 0),
                                stop=(k == KT - 1),
                            )
                    osb = opool.tile(
                        [P, olen], F32, tag="o", name="o",
                        padded_shape=[P, OBLK],
                    )
                    for b, blen in enumerate(banks):
                        nc.vector.tensor_add(
                            osb[:, ts(b, blen)],
                            accs[b][:],
                            bias_sb[:, o0 + b * blen : o0 + (b + 1) * blen],
                        )
                    nc.sync.dma_start(
                        out.ap()[ts(t, P), o0 : o0 + olen], osb[:]
                    )

    nc.compile()
    return nc


def kernel(x, weight, bias):
    global _NC, LAST_RESULT
    if _NC is None:
        _NC = _build_nc()

    X = np.ascontiguousarray(x.reshape(B * S, DIN))
    wt = weight.T.astype(ml_dtypes.bfloat16)          # [k, o] bf16
    bias_rep = np.ascontiguousarray(
        np.broadcast_to(bias.astype(np.float32), (P, DOUT))
    )
    in_maps = []
    for c in range(NCORES):
        xc = X[c * T : (c + 1) * T].astype(ml_dtypes.bfloat16)
        # [t-tile, p(=k%128), ks, i(=token%128)]
        xt_c = np.ascontiguousarray(
            xc.reshape(NT, P, KT, P).transpose(0, 3, 2, 1)
        )
        in_maps.append({"xt": xt_c, "wt": wt, "bias_rep": bias_rep})

    last_err = None
    for _attempt in range(2):
        try:
            res = run_bass_kernel_spmd(_NC, in_maps, list(range(NCORES)))
            break
        except Exception as e:  # transient NRT device errors: retry once
            last_err = e
    else:
        raise last_err
    LAST_RESULT = res

    out = np.concatenate([res.results[c]["out"] for c in range(NCORES)], axis=0)
    return out.reshape(B, S, DOUT).astype(np.float32, copy=False)

